# revision 1
# baseline (speedup 1.0000x reference)
"""v12: host-prepped bf16 inputs (xT pre-transposed, w split/converted) kill
all on-device conversions/transposes; single software-pipelined attention
stream (lag-1 scores->exp->AV) with qkv/v/proj emission chunks pumped as PE
filler so the exp latency is never exposed; bias add moved to DVE; PE warmup
matmuls ramp the clock before the first real work.
"""
import sys

sys.path.insert(0, "/opt/trn_rl_repo")

import numpy as np

N = 1024
D = 1024
H = 16
HD = 64
SCALE = HD ** -0.5
P = 128
NT = N // P          # 8 token tiles
DTn = D // P         # 8 dim tiles
HC = 512             # half-row chunk (one psum bank of fp32)
VW = 65              # v width per head: 64 dims + ones column (denominator)

_CACHE: dict = {}


def _build_nc(repeat=1):
    import concourse.bass as bass  # noqa: F401
    import concourse.tile as tile
    from concourse import bacc, mybir
    from contextlib import ExitStack

    fp32 = mybir.dt.float32
    bf16 = mybir.dt.bfloat16
    EXP = mybir.ActivationFunctionType.Exp

    nc = bacc.Bacc("TRN2", debug=False, num_devices=8)
    xT_d = nc.dram_tensor("xT", [D, N], bf16, kind="ExternalInput").ap()
    wq_d = nc.dram_tensor("wq", [D, D], bf16, kind="ExternalInput").ap()
    wk_d = nc.dram_tensor("wk", [D, D], bf16, kind="ExternalInput").ap()
    wv_d = nc.dram_tensor("wv", [D, D], bf16, kind="ExternalInput").ap()
    wp_d = nc.dram_tensor("wp", [D, D], bf16, kind="ExternalInput").ap()
    b_d = nc.dram_tensor("b_proj", [1, D], bf16, kind="ExternalInput").ap()
    out_d = nc.dram_tensor("out", [N, D], bf16, kind="ExternalOutput").ap()

    with tile.TileContext(nc) as tc, ExitStack() as ctx:
        constp = ctx.enter_context(tc.tile_pool(name="const", bufs=1))
        xTp = ctx.enter_context(tc.tile_pool(name="xT", bufs=DTn))
        wqp = ctx.enter_context(tc.tile_pool(name="wq", bufs=DTn))
        wkp = ctx.enter_context(tc.tile_pool(name="wk", bufs=DTn))
        wvp = ctx.enter_context(tc.tile_pool(name="wv", bufs=DTn))
        wpp = ctx.enter_context(tc.tile_pool(name="wp", bufs=DTn))
        qkTp = ctx.enter_context(tc.tile_pool(name="qkT", bufs=2 * NT))
        vp = ctx.enter_context(tc.tile_pool(name="vsb", bufs=2 * NT))
        eTp = ctx.enter_context(tc.tile_pool(name="eT", bufs=4))
        attnp = ctx.enter_context(tc.tile_pool(name="attnT", bufs=2 * NT))
        avsp = ctx.enter_context(tc.tile_pool(name="avs", bufs=3))
        recp = ctx.enter_context(tc.tile_pool(name="rec", bufs=3))
        rbp = ctx.enter_context(tc.tile_pool(name="rb", bufs=3))
        ysbp = ctx.enter_context(tc.tile_pool(name="ysb", bufs=6))
        psSp = ctx.enter_context(tc.tile_pool(name="psS", bufs=2, space="PSUM"))
        psAp = ctx.enter_context(tc.tile_pool(name="psA", bufs=2, space="PSUM"))
        psEp = ctx.enter_context(tc.tile_pool(name="psE", bufs=2, space="PSUM"))

        # ---- warmup (once): ramp the PE clock before the first real matmul
        warm = constp.tile([P, HC], bf16, name="warm", tag="warm")
        nc.gpsimd.memset(warm[:], 0.0)
        for _w in range(3):
            pw = psEp.tile([P, HC], fp32, name="pw", tag="em")
            nc.tensor.matmul(pw[:], lhsT=warm[:, 0:P], rhs=warm[:],
                             start=True, stop=True)

        # ---- rep-invariant loads: bias + weights stay resident in SBUF ----
        b_sb = constp.tile([1, D], bf16, name="b_sb", tag="b_sb")
        nc.sync.dma_start(b_sb[:], b_d[:])
        bias_bc = constp.tile([P, D], bf16, name="bias_bc", tag="bias_bc")
        nc.gpsimd.partition_broadcast(bias_bc[:], b_sb[:])
        wqs = [wqp.tile([P, D], bf16, name=f"wq{t}", tag="wq")
               for t in range(DTn)]
        wks = [wkp.tile([P, D], bf16, name=f"wk{t}", tag="wk")
               for t in range(DTn)]
        wvs = [wvp.tile([P, D], bf16, name=f"wv{t}", tag="wv")
               for t in range(DTn)]
        wps = [wpp.tile([P, D], bf16, name=f"wp{t}", tag="wp")
               for t in range(DTn)]

        for _rep in range(repeat):
            # ---- per-rep input DMAs (order = arrival priority) ----
            xT = [xTp.tile([P, N], bf16, name=f"xT{t}", tag="xT")
                  for t in range(DTn)]
            for t in range(DTn):
                nc.sync.dma_start(xT[t][:], xT_d[t * P:(t + 1) * P, :])
            if _rep == 0:
                for tiles, src in ((wqs, wq_d), (wks, wk_d), (wvs, wv_d),
                                   (wps, wp_d)):
                    for t in range(DTn):
                        nc.sync.dma_start(tiles[t][:],
                                          src[t * P:(t + 1) * P, :])

            # ---- persistent result tiles ----
            qkT = [qkTp.tile([P, N], bf16, name=f"qkT{ft}", tag="qkT")
                   for ft in range(2 * NT)]
            vsb = [vp.tile([P, H * VW], bf16, name=f"v{jt}", tag="v")
                   for jt in range(NT)]
            attnT = [[attnp.tile([P, HC], bf16, name=f"attnT{c}_{q}",
                                 tag="attnT") for q in range(NT)]
                     for c in range(2)]

            # ---- emission task generators (PE filler work) ----
            def gen_qkT(ft):
                wt = wqs if ft < NT else wks
                col = (ft % NT) * P
                dst = qkT[ft]
                for ic in range(2):
                    ps = psEp.tile([P, HC], fp32, name="em_t", tag="em")
                    for dt in range(DTn):
                        nc.tensor.matmul(
                            ps[:], lhsT=wt[dt][:, col:col + P],
                            rhs=xT[dt][:, ic * HC:(ic + 1) * HC],
                            start=(dt == 0), stop=(dt == DTn - 1))
                        yield
                    nc.vector.tensor_copy(dst[:, ic * HC:(ic + 1) * HC],
                                          ps[:])
                    yield

            def gen_v(jt):
                v = vsb[jt]
                nc.gpsimd.memset(v[:], 1.0)
                vv = v[:].rearrange("p (h c) -> p h c", c=VW)
                for dc in range(2):
                    ps = psEp.tile([P, HC], fp32, name="em_t", tag="em")
                    for dt in range(DTn):
                        nc.tensor.matmul(
                            ps[:], lhsT=xT[dt][:, jt * P:(jt + 1) * P],
                            rhs=wvs[dt][:, dc * HC:(dc + 1) * HC],
                            start=(dt == 0), stop=(dt == DTn - 1))
                        yield
                    pv = ps[:].rearrange("p (h c) -> p h c", c=HD)
                    nc.vector.tensor_copy(vv[:, dc * 8:(dc + 1) * 8, 0:HD],
                                          pv)
                    yield

            y_of = {}

            def gen_proj(it, dts=range(DTn), fixup=False):
                # fixup=False with dts=0..6 accumulates all pairs except the
                # last into y (with bias); the 1-matmul fixup adds pair 7
                # after its epilogue lands, so the tail never waits long.
                icb = it // 4
                ic2 = it % 4
                if it in y_of:
                    y = y_of[it]
                else:
                    y = ysbp.tile([P, N], bf16, name="y_t", tag="y")
                    y_of[it] = y
                dts = list(dts)
                for fc in range(2):
                    # fixup runs after the SC stream ends: use the idle psS
                    # banks so the matmul never waits on the DVE-backlogged
                    # psE evacuations at the tail.
                    if fixup:
                        ps = psSp.tile([P, HC], fp32, name="fix_t", tag="pss")
                    else:
                        ps = psEp.tile([P, HC], fp32, name="em_t", tag="em")
                    for i, dt in enumerate(dts):
                        nc.tensor.matmul(
                            ps[:],
                            lhsT=attnT[icb][dt][:, ic2 * P:(ic2 + 1) * P],
                            rhs=wps[dt][:, fc * HC:(fc + 1) * HC],
                            start=(i == 0), stop=(i == len(dts) - 1))
                        yield
                    ysl = y[:, fc * HC:(fc + 1) * HC]
                    if fixup:
                        nc.vector.tensor_add(ysl, ysl, ps[:])
                    else:
                        nc.vector.tensor_add(ysl, ps[:],
                                             bias_bc[:, fc * HC:(fc + 1) * HC])
                    yield
                if fixup or len(dts) == DTn:
                    nc.sync.dma_start(out_d[it * P:(it + 1) * P, :], y[:])
                    yield

            tasks = []
            done_gens = [0]

            def pump(n):
                while n > 0 and tasks:
                    try:
                        next(tasks[0])
                        n -= 1
                    except StopIteration:
                        tasks.pop(0)
                        done_gens[0] += 1

            def drain_until(gens):
                while done_gens[0] < gens and tasks:
                    pump(1 << 30)

            def drain():
                while tasks:
                    pump(1 << 30)

            # ---- attention stream ----
            e_of = {}
            av_of = {}

            def decode(g):
                icb, g2 = divmod(g, 64)
                hp, jt = divmod(g2, NT)
                return hp, icb, jt

            def SC(g):
                hp, icb, jt = decode(g)
                qa, ka = qkT[hp], qkT[NT + hp]
                i0 = icb * HC
                ps = psSp.tile([P, N], fp32, name="pss_t", tag="pss")
                nc.tensor.matmul(ps[:, 0:HC],
                                 lhsT=ka[0:HD, jt * P:(jt + 1) * P],
                                 rhs=qa[0:HD, i0:i0 + HC],
                                 start=True, stop=True)
                nc.tensor.matmul(ps[:, HC:N],
                                 lhsT=ka[HD:P, jt * P:(jt + 1) * P],
                                 rhs=qa[HD:P, i0:i0 + HC],
                                 start=True, stop=True)
                e = eTp.tile([P, N], bf16, name="e_t", tag="e")
                nc.scalar.activation(e[:], ps[:], EXP, scale=SCALE)
                e_of[g] = e

            def AV(g):
                # head b's accumulator starts one step late (its jt=0 term is
                # replayed at jt=1 from the still-live e tile) so the two psA
                # slot requests at a pair crossing are staggered.
                hp, icb, jt = decode(g)
                ha, hb = 2 * hp, 2 * hp + 1
                e = e_of[g]
                if jt == 0:
                    av_of[hp] = {ha: psAp.tile([VW, HC], fp32,
                                               name=f"av{ha}", tag="av")}
                    nc.tensor.matmul(av_of[hp][ha][:],
                                     lhsT=vsb[0][:, ha * VW:(ha + 1) * VW],
                                     rhs=e[:, 0:HC], start=True, stop=False)
                    return
                nc.tensor.matmul(av_of[hp][ha][:],
                                 lhsT=vsb[jt][:, ha * VW:(ha + 1) * VW],
                                 rhs=e[:, 0:HC],
                                 start=False, stop=(jt == NT - 1))
                if jt == 1:
                    av_of[hp][hb] = psAp.tile([VW, HC], fp32,
                                              name=f"av{hb}", tag="av")
                    e0 = e_of.pop(g - 1)
                    nc.tensor.matmul(av_of[hp][hb][:],
                                     lhsT=vsb[0][:, hb * VW:(hb + 1) * VW],
                                     rhs=e0[:, HC:N], start=True, stop=False)
                nc.tensor.matmul(av_of[hp][hb][:],
                                 lhsT=vsb[jt][:, hb * VW:(hb + 1) * VW],
                                 rhs=e[:, HC:N],
                                 start=False, stop=(jt == NT - 1))
                e_of.pop(g)

            def epilogue_one(hp, icb, h, av):
                off = 0 if h % 2 == 0 else HD
                avs = avsp.tile([VW, HC], bf16, name=f"avs{h}", tag="avs")
                recf = recp.tile([1, HC], bf16, name=f"recf{h}", tag="recf")
                nc.vector.tensor_copy(avs[:], av[:])
                with nc.allow_low_precision(reason="bf16 softmax denom"):
                    nc.vector.reciprocal(recf[:], avs[HD:VW, :])
                rb = rbp.tile([HD, HC], bf16, name=f"rb{h}", tag="rb")
                nc.gpsimd.partition_broadcast(rb[:], recf[:])
                nc.vector.tensor_mul(attnT[icb][hp][off:off + HD, :],
                                     avs[0:HD, :], rb[:])

            # pre-phase: q/k for pairs 0-1, all of V (PE is DMA-gated here)
            tasks.append(gen_qkT(0))
            tasks.append(gen_qkT(NT))
            for jt in range(NT):
                tasks.append(gen_v(jt))
            tasks.append(gen_qkT(1))
            tasks.append(gen_qkT(NT + 1))
            drain()

            done_gens[0] = 0
            for p in range(2, NT):
                tasks.append(gen_qkT(p))
                tasks.append(gen_qkT(NT + p))

            SC(0)
            pending_epi = None
            for g in range(128):
                hp, icb, jt = decode(g)
                if g + 1 < 128:
                    SC(g + 1)
                if pending_epi is not None:
                    epilogue_one(*pending_epi)
                    pending_epi = None
                pump(4 if g < 64 else 2)
                AV(g)
                if jt == NT - 1:
                    # head a's epilogue now; head b's one step later so the
                    # DVE burst doesn't delay the psA/psE recycling the next
                    # pair's matmuls wait on. Last pair: both immediately.
                    av = av_of.pop(hp)
                    epilogue_one(hp, icb, 2 * hp, av[2 * hp])
                    if g == 127:
                        epilogue_one(hp, icb, 2 * hp + 1, av[2 * hp + 1])
                    else:
                        pending_epi = (hp, icb, 2 * hp + 1, av[2 * hp + 1])
                if g == 64:
                    for it in range(4):
                        tasks.append(gen_proj(it))
                if g == 104:
                    for it in range(4, NT):
                        tasks.append(gen_proj(it, dts=range(DTn - 3)))
            for it in range(4, NT):
                tasks.append(gen_proj(it, dts=[DTn - 3, DTn - 2, DTn - 1],
                                      fixup=True))
            drain()

    nc.compile()
    return nc


def get_nc():
    if "nc" not in _CACHE:
        _CACHE["nc"] = _build_nc()
    return _CACHE["nc"]


def make_in_maps(x, w_qkv, w_proj, b_proj):
    import ml_dtypes

    bf = ml_dtypes.bfloat16
    w = np.asarray(w_qkv, np.float32)
    wq = np.ascontiguousarray(w[:, 0:D]).astype(bf)
    wk = np.ascontiguousarray(w[:, D:2 * D]).astype(bf)
    wv = np.ascontiguousarray(w[:, 2 * D:3 * D]).astype(bf)
    wp = np.ascontiguousarray(np.asarray(w_proj, np.float32)).astype(bf)
    b2 = np.asarray(b_proj, np.float32).reshape(1, D).astype(bf)
    x = np.asarray(x, np.float32)
    maps = []
    for i in range(8):
        xT = np.ascontiguousarray(x[i].T).astype(bf)
        maps.append({"xT": xT, "wq": wq, "wk": wk, "wv": wv, "wp": wp,
                     "b_proj": b2})
    return maps


def kernel(x, w_qkv, w_proj, b_proj):
    from concourse import bass_utils

    nc = get_nc()
    in_maps = make_in_maps(x, w_qkv, w_proj, b_proj)
    res = bass_utils.run_bass_kernel_spmd(nc, in_maps, core_ids=list(range(8)))
    return np.stack(
        [np.asarray(res.results[i]["out"]).astype(np.float32)
         for i in range(8)], axis=0)



# revision 4
# speedup vs baseline: 1.1740x; 1.1740x over previous
"""v13: attention stream restructured around PE tile packing. AV drops the
M=65 ones-column design: the two heads of a pair run as col-packed M=64
matmuls (tile_position col groups 0-1 / 2-3, concurrent), and the softmax
denominators come from 4-way col-packed M=1 ones-matmuls accumulating into
PSUM rows 0/32/64/96 (even/odd key-tile split, merged via tiny SBUF DMAs in
the epilogue). Attention cost falls from 3 to 2.5 PE passes per (pair,
key-tile): -32k PE cycles. Emission/proj pipeline and host-side prep are
unchanged from v12.
"""
import sys

sys.path.insert(0, "/opt/trn_rl_repo")

import numpy as np

N = 1024
D = 1024
H = 16
HD = 64
SCALE = HD ** -0.5
P = 128
NT = N // P          # 8 token tiles
DTn = D // P         # 8 dim tiles
HC = 512             # half-row chunk (one psum bank of fp32)

_CACHE: dict = {}


def _build_nc(repeat=1):
    import concourse.bass as bass  # noqa: F401
    import concourse.tile as tile
    from concourse import bacc, mybir
    from contextlib import ExitStack

    fp32 = mybir.dt.float32
    bf16 = mybir.dt.bfloat16
    EXP = mybir.ActivationFunctionType.Exp

    nc = bacc.Bacc("TRN2", debug=False, num_devices=8)
    xT_d = nc.dram_tensor("xT", [D, N], bf16, kind="ExternalInput").ap()
    wq_d = nc.dram_tensor("wq", [D, D], bf16, kind="ExternalInput").ap()
    wk_d = nc.dram_tensor("wk", [D, D], bf16, kind="ExternalInput").ap()
    wv_d = nc.dram_tensor("wv", [D, D], bf16, kind="ExternalInput").ap()
    wp_d = nc.dram_tensor("wp", [D, D], bf16, kind="ExternalInput").ap()
    b_d = nc.dram_tensor("b_proj", [1, D], bf16, kind="ExternalInput").ap()
    out_d = nc.dram_tensor("out", [N, D], bf16, kind="ExternalOutput").ap()

    with tile.TileContext(nc) as tc, ExitStack() as ctx:
        constp = ctx.enter_context(tc.tile_pool(name="const", bufs=1))
        xTp = ctx.enter_context(tc.tile_pool(name="xT", bufs=DTn))
        wqp = ctx.enter_context(tc.tile_pool(name="wq", bufs=DTn))
        wkp = ctx.enter_context(tc.tile_pool(name="wk", bufs=DTn))
        wvp = ctx.enter_context(tc.tile_pool(name="wv", bufs=DTn))
        wpp = ctx.enter_context(tc.tile_pool(name="wp", bufs=DTn))
        qkTp = ctx.enter_context(tc.tile_pool(name="qkT", bufs=2 * NT))
        vp = ctx.enter_context(tc.tile_pool(name="vsb", bufs=NT + 2))
        eTp = ctx.enter_context(tc.tile_pool(name="eT", bufs=6))
        attnp = ctx.enter_context(tc.tile_pool(name="attnT", bufs=2 * NT))
        dcpp = ctx.enter_context(tc.tile_pool(name="dcp", bufs=2))
        tmvp = ctx.enter_context(tc.tile_pool(name="tmv", bufs=6))
        dsump = ctx.enter_context(tc.tile_pool(name="dsum", bufs=4))
        recp = ctx.enter_context(tc.tile_pool(name="rec", bufs=4))
        rbap = ctx.enter_context(tc.tile_pool(name="rba", bufs=2))
        rbbp = ctx.enter_context(tc.tile_pool(name="rbb", bufs=2))
        ysbp = ctx.enter_context(tc.tile_pool(name="ysb", bufs=6))
        psSp = ctx.enter_context(tc.tile_pool(name="psS", bufs=2, space="PSUM"))
        psAp = ctx.enter_context(tc.tile_pool(name="psA", bufs=2, space="PSUM"))
        psDp = ctx.enter_context(tc.tile_pool(name="psD", bufs=2, space="PSUM"))
        psEp = ctx.enter_context(tc.tile_pool(name="psE", bufs=2, space="PSUM"))

        # ---- warmup (once): ramp the PE clock before the first real matmul
        warm = constp.tile([P, HC], bf16, name="warm", tag="warm")
        nc.gpsimd.memset(warm[:], 0.0)
        for _w in range(3):
            pw = psEp.tile([P, HC], fp32, name="pw", tag="em")
            nc.tensor.matmul(pw[:], lhsT=warm[:, 0:P], rhs=warm[:],
                             start=True, stop=True)

        # ---- rep-invariant loads: bias + weights + ones stay resident ----
        b_sb = constp.tile([1, D], bf16, name="b_sb", tag="b_sb")
        nc.sync.dma_start(b_sb[:], b_d[:])
        bias_bc = constp.tile([P, D], bf16, name="bias_bc", tag="bias_bc")
        nc.gpsimd.partition_broadcast(bias_bc[:], b_sb[:])
        ones_sb = constp.tile([P, 1], bf16, name="ones_sb", tag="ones")
        nc.gpsimd.memset(ones_sb[:], 1.0)
        wqs = [wqp.tile([P, D], bf16, name=f"wq{t}", tag="wq")
               for t in range(DTn)]
        wks = [wkp.tile([P, D], bf16, name=f"wk{t}", tag="wk")
               for t in range(DTn)]
        wvs = [wvp.tile([P, D], bf16, name=f"wv{t}", tag="wv")
               for t in range(DTn)]
        wps = [wpp.tile([P, D], bf16, name=f"wp{t}", tag="wp")
               for t in range(DTn)]

        for _rep in range(repeat):
            # ---- per-rep input DMAs (order = arrival priority) ----
            xT = [xTp.tile([P, N], bf16, name=f"xT{t}", tag="xT")
                  for t in range(DTn)]
            for t in range(DTn):
                nc.sync.dma_start(xT[t][:], xT_d[t * P:(t + 1) * P, :])
            if _rep == 0:
                for tiles, src in ((wqs, wq_d), (wks, wk_d), (wvs, wv_d),
                                   (wps, wp_d)):
                    for t in range(DTn):
                        nc.sync.dma_start(tiles[t][:],
                                          src[t * P:(t + 1) * P, :])

            # ---- persistent result tiles ----
            qkT = [qkTp.tile([P, N], bf16, name=f"qkT{ft}", tag="qkT")
                   for ft in range(2 * NT)]
            vsb = [vp.tile([P, D], bf16, name=f"v{jt}", tag="v")
                   for jt in range(NT)]
            attnT = [[attnp.tile([P, HC], bf16, name=f"attnT{c}_{q}",
                                 tag="attnT") for q in range(NT)]
                     for c in range(2)]

            # ---- emission task generators (PE filler work) ----
            def gen_qkT(ft):
                wt = wqs if ft < NT else wks
                col = (ft % NT) * P
                dst = qkT[ft]
                for ic in range(2):
                    ps = psEp.tile([P, HC], fp32, name="em_t", tag="em")
                    for dt in range(DTn):
                        nc.tensor.matmul(
                            ps[:], lhsT=wt[dt][:, col:col + P],
                            rhs=xT[dt][:, ic * HC:(ic + 1) * HC],
                            start=(dt == 0), stop=(dt == DTn - 1))
                        yield
                    nc.vector.tensor_copy(dst[:, ic * HC:(ic + 1) * HC],
                                          ps[:])
                    yield

            def gen_v(jt):
                v = vsb[jt]
                for dc in range(2):
                    ps = psEp.tile([P, HC], fp32, name="em_t", tag="em")
                    for dt in range(DTn):
                        nc.tensor.matmul(
                            ps[:], lhsT=xT[dt][:, jt * P:(jt + 1) * P],
                            rhs=wvs[dt][:, dc * HC:(dc + 1) * HC],
                            start=(dt == 0), stop=(dt == DTn - 1))
                        yield
                    nc.vector.tensor_copy(v[:, dc * HC:(dc + 1) * HC], ps[:])
                    yield

            y_of = {}

            def gen_proj(it, dts=range(DTn), fixup=False):
                # fixup=False with dts=0..4 accumulates all pairs except the
                # last into y (with bias); the 3-matmul fixup adds the rest
                # after their epilogues land, so the tail never waits long.
                icb = it // 4
                ic2 = it % 4
                if it in y_of:
                    y = y_of[it]
                else:
                    y = ysbp.tile([P, N], bf16, name="y_t", tag="y")
                    y_of[it] = y
                dts = list(dts)
                for fc in range(2):
                    # fixup runs after the SC stream ends: use the idle psS
                    # banks so the matmul never waits on the DVE-backlogged
                    # psE evacuations at the tail.
                    if fixup:
                        ps = psSp.tile([P, HC], fp32, name="fix_t", tag="pss")
                    else:
                        ps = psEp.tile([P, HC], fp32, name="em_t", tag="em")
                    for i, dt in enumerate(dts):
                        nc.tensor.matmul(
                            ps[:],
                            lhsT=attnT[icb][dt][:, ic2 * P:(ic2 + 1) * P],
                            rhs=wps[dt][:, fc * HC:(fc + 1) * HC],
                            start=(i == 0), stop=(i == len(dts) - 1))
                        yield
                    ysl = y[:, fc * HC:(fc + 1) * HC]
                    if fixup:
                        nc.vector.tensor_add(ysl, ysl, ps[:])
                    else:
                        nc.vector.tensor_add(ysl, ps[:],
                                             bias_bc[:, fc * HC:(fc + 1) * HC])
                    yield
                if fixup or len(dts) == DTn:
                    nc.sync.dma_start(out_d[it * P:(it + 1) * P, :], y[:])
                    yield

            tasks = []
            done_gens = [0]

            def pump(n):
                while n > 0 and tasks:
                    try:
                        next(tasks[0])
                        n -= 1
                    except StopIteration:
                        tasks.pop(0)
                        done_gens[0] += 1

            def drain():
                while tasks:
                    pump(1 << 30)

            # ---- attention stream ----
            e_of = {}
            av_of = {}
            den_of = {}

            def decode(g):
                icb, g2 = divmod(g, 64)
                hp, jt = divmod(g2, NT)
                return hp, icb, jt

            def SC(g):
                # two K=64 matmuls row-packed (rows 0:64 / 64:128 of the PE
                # array run concurrently); each writes its own psum bank.
                hp, icb, jt = decode(g)
                qa, ka = qkT[hp], qkT[NT + hp]
                i0 = icb * HC
                pss = []
                for h in range(2):
                    ps = psSp.tile([P, HC], fp32, name="pss_t", tag="pss")
                    nc.tensor.matmul(
                        ps[:], lhsT=ka[h * HD:(h + 1) * HD, jt * P:(jt + 1) * P],
                        rhs=qa[h * HD:(h + 1) * HD, i0:i0 + HC],
                        start=True, stop=True)
                    pss.append(ps)
                es = []
                for h in range(2):
                    e = eTp.tile([P, HC], bf16, name="e_t", tag="e")
                    nc.scalar.activation(e[:], pss[h][:], EXP, scale=SCALE)
                    es.append(e)
                e_of[g] = es

            def AV(g):
                # col-packed pair: head a -> psum partitions 0:64 (col groups
                # 0-1), head b -> 64:128 (col groups 2-3); concurrent on PE.
                hp, icb, jt = decode(g)
                ea, eb = e_of[g]
                key = (icb, hp)
                if jt == 0:
                    av_of[key] = psAp.tile([P, HC], fp32, name="av_t",
                                           tag="av")
                av = av_of[key]
                ha, hb = 2 * hp, 2 * hp + 1
                nc.tensor.matmul(av[0:HD, :],
                                 lhsT=vsb[jt][:, ha * HD:(ha + 1) * HD],
                                 rhs=ea[:],
                                 start=(jt == 0), stop=(jt == NT - 1))
                nc.tensor.matmul(av[HD:P, :],
                                 lhsT=vsb[jt][:, hb * HD:(hb + 1) * HD],
                                 rhs=eb[:],
                                 start=(jt == 0), stop=(jt == NT - 1))

            def DEN(g):
                # 4-way col-packed M=1 ones-matmuls: denominators for heads
                # a/b of key tiles jt-1 (even) and jt (odd) accumulate into
                # psum rows 0/32/64/96 concurrently (col groups 0/1/2/3).
                hp, icb, jt = decode(g)
                key = (icb, hp)
                if jt == 1:
                    den_of[key] = psDp.tile([P, HC], fp32, name="den_t",
                                            tag="den")
                dp = den_of[key]
                e0a, e0b = e_of.pop(g - 1)
                e1a, e1b = e_of.pop(g)
                start = jt == 1
                stop = jt == NT - 1
                nc.tensor.matmul(dp[0:1, :], lhsT=ones_sb[:], rhs=e0a[:],
                                 start=start, stop=stop)
                nc.tensor.matmul(dp[32:33, :], lhsT=ones_sb[:], rhs=e0b[:],
                                 start=start, stop=stop)
                nc.tensor.matmul(dp[64:65, :], lhsT=ones_sb[:], rhs=e1a[:],
                                 start=start, stop=stop)
                nc.tensor.matmul(dp[96:97, :], lhsT=ones_sb[:], rhs=e1b[:],
                                 start=start, stop=stop, tile_position=(0, 96))

            def epilogue(key):
                # merge even/odd denominator rows (tiny SBUF DMAs move rows
                # 32/64/96 to partition 0), reciprocal, broadcast, normalize
                # av into attnT. b-half uses the lower half of a full
                # 128-partition broadcast (dst partition offsets are not
                # supported by partition_broadcast).
                icb, hp = key
                av = av_of.pop(key)
                dp = den_of.pop(key)
                dcp = dcpp.tile([97, HC], fp32, name="dcp_t", tag="dcp")
                nc.vector.tensor_copy(dcp[:], dp[0:97, :])
                t32 = tmvp.tile([1, HC], fp32, name="t32", tag="tmv")
                t64 = tmvp.tile([1, HC], fp32, name="t64", tag="tmv")
                t96 = tmvp.tile([1, HC], fp32, name="t96", tag="tmv")
                nc.sync.dma_start(t32[:], dcp[32:33, :])
                nc.sync.dma_start(t64[:], dcp[64:65, :])
                nc.sync.dma_start(t96[:], dcp[96:97, :])
                da = dsump.tile([1, HC], fp32, name="da", tag="dsum")
                db = dsump.tile([1, HC], fp32, name="db", tag="dsum")
                nc.vector.tensor_add(da[:], dcp[0:1, :], t64[:])
                nc.vector.tensor_add(db[:], t32[:], t96[:])
                ra = recp.tile([1, HC], bf16, name="ra", tag="rec")
                rb_ = recp.tile([1, HC], bf16, name="rb", tag="rec")
                with nc.allow_low_precision(reason="bf16 softmax denom"):
                    nc.vector.reciprocal(ra[:], da[:])
                    nc.vector.reciprocal(rb_[:], db[:])
                rba = rbap.tile([HD, HC], bf16, name="rba_t", tag="rba")
                nc.gpsimd.partition_broadcast(rba[:], ra[:])
                rbb = rbbp.tile([P, HC], bf16, name="rbb_t", tag="rbb")
                nc.gpsimd.partition_broadcast(rbb[:], rb_[:])
                dst = attnT[icb][hp]
                nc.vector.tensor_mul(dst[0:HD, :], av[0:HD, :], rba[:])
                nc.vector.tensor_mul(dst[HD:P, :], av[HD:P, :],
                                     rbb[HD:P, :])

            # pre-phase: q/k for pairs 0-1, all of V (PE is DMA-gated here)
            tasks.append(gen_qkT(0))
            tasks.append(gen_qkT(NT))
            for jt in range(NT):
                tasks.append(gen_v(jt))
            tasks.append(gen_qkT(1))
            tasks.append(gen_qkT(NT + 1))
            drain()

            done_gens[0] = 0
            for p in range(2, NT):
                tasks.append(gen_qkT(p))
                tasks.append(gen_qkT(NT + p))

            SC(0)
            pending_epi = None
            for g in range(128):
                hp, icb, jt = decode(g)
                if g + 1 < 128:
                    SC(g + 1)
                if pending_epi is not None:
                    epilogue(pending_epi)
                    pending_epi = None
                pump(4 if g < 64 else 2)
                AV(g)
                if jt % 2 == 1:
                    DEN(g)
                if jt == NT - 1:
                    if g == 127:
                        epilogue((icb, hp))
                    else:
                        pending_epi = (icb, hp)
                if g == 64:
                    for it in range(4):
                        tasks.append(gen_proj(it))
                if g == 104:
                    for it in range(4, NT):
                        tasks.append(gen_proj(it, dts=range(DTn - 3)))
            for it in range(4, NT):
                tasks.append(gen_proj(it, dts=[DTn - 3, DTn - 2, DTn - 1],
                                      fixup=True))
            drain()

    nc.compile()
    return nc


def get_nc():
    if "nc" not in _CACHE:
        _CACHE["nc"] = _build_nc()
    return _CACHE["nc"]


def make_in_maps(x, w_qkv, w_proj, b_proj):
    import ml_dtypes

    bf = ml_dtypes.bfloat16
    w = np.asarray(w_qkv, np.float32)
    wq = np.ascontiguousarray(w[:, 0:D]).astype(bf)
    wk = np.ascontiguousarray(w[:, D:2 * D]).astype(bf)
    wv = np.ascontiguousarray(w[:, 2 * D:3 * D]).astype(bf)
    wp = np.ascontiguousarray(np.asarray(w_proj, np.float32)).astype(bf)
    b2 = np.asarray(b_proj, np.float32).reshape(1, D).astype(bf)
    x = np.asarray(x, np.float32)
    maps = []
    for i in range(8):
        xT = np.ascontiguousarray(x[i].T).astype(bf)
        maps.append({"xT": xT, "wq": wq, "wk": wk, "wv": wv, "wp": wp,
                     "b_proj": b2})
    return maps


def kernel(x, w_qkv, w_proj, b_proj):
    from concourse import bass_utils

    nc = get_nc()
    in_maps = make_in_maps(x, w_qkv, w_proj, b_proj)
    res = bass_utils.run_bass_kernel_spmd(nc, in_maps, core_ids=list(range(8)))
    return np.stack(
        [np.asarray(res.results[i]["out"]).astype(np.float32)
         for i in range(8)], axis=0)
